# revision 30
# baseline (speedup 1.0000x reference)
"""Multi-head self-attention (causal) Trainium2 kernel, 8-way sharded.

Sharding: core c handles batch b = c//4 and head group g = c%4 (4 of 16
heads). Each core computes q/k/v projections for its head slice, causal
softmax attention, and a partial o_proj ([2048, 1024] bf16); the host
sums the 4 partials per batch in f32.

Layouts (per core):
  xT    [1024, 2048]  x[b].T            (d_model on partitions)
  wqT   [1024,  256]  Wq[g*256:(g+1)*256, :].T      (same for wk/wv)
  woT   [ 256, 1024]  Wo[:, g*256:(g+1)*256].T
  utri  [ 128,  128]  -200 where p > j (strict lower = masked keys)
  ident [ 128,  128]  identity (stationary for the causal-bias matmul)

Everything on-chip is bf16 (PSUM accumulation stays fp32): bf16 matmuls
stream 1 col/cycle at any width (f32r pays 2-4x below 256 cols), so the
diagonal chunks run at exact 128-col granularity with a single 128-wide
causal-bias matmul (ident.T @ utri = -200 above the diagonal; exp -> 0).

q/k projections run with 4 PSUM groups per slab so matmuls start as soon
as the first x chunk lands. kT/qT are stored head-major stacked two
heads per partition column; S matmuls contract over K=64 partition
slices so no zero padding is needed.

Startup: the first matmuls need only wq + x slab0, so those four DMAs
issue first on four separate queues (sync/scalar HWDGE + gpsimd/vector);
everything else queues behind. Attention pipeline runs at depth 3
(S(kt) issues, O(kt-2) pops) so the ACT exp latency (~1.15us per kt
chunk) stays off the PE critical path. V carries an appended ones column
so PSUM partition 64 accumulates the softmax sums; normalization is a
custom-DVE reciprocal_approx_fast + GpSimd partition_broadcast + DVE
multiply. o_proj chunks are [128, 1024] with a single out DMA each.
"""

import ml_dtypes
import numpy as np

import concourse.bass as bass
import concourse.mybir as mybir
import concourse.tile as tile
from concourse import bacc
from concourse.bass_utils import run_bass_kernel_spmd

P = 128
S = 2048  # sequence length
DM = 1024  # d_model
HD = 64  # head dim
NH_CORE = 4  # heads per core
HSL = NH_CORE * HD  # head slice width = 256
QC = 512  # query chunk
N_QC = S // QC  # 4
N_KT = S // P  # 16 key tiles
KO = DM // P  # 8 k-tiles over d_model

f32 = mybir.dt.float32
bf16 = mybir.dt.bfloat16
f8 = mybir.dt.float8e4

EXP_SCALE = 0.125
UTRI_VAL = -200.0

_CACHED = {}


def build_program():
    nc = bacc.Bacc("TRN2", target_bir_lowering=False, debug=False)
    # all inputs host-prearranged into SBUF tile layouts so every DMA line
    # is one long contiguous read per partition (no strided descriptors).
    xS = nc.declare_dram_parameter("xS", [N_QC, P, KO, QC], bf16, isOutput=False)
    wqT = nc.declare_dram_parameter("wqT", [P, KO, HSL], bf16, isOutput=False)
    wkT = nc.declare_dram_parameter("wkT", [P, KO, HSL], bf16, isOutput=False)
    wvT = nc.declare_dram_parameter("wvT", [P, KO, HSL], bf16, isOutput=False)
    woT = nc.declare_dram_parameter("woT", [P, 2, DM], bf16, isOutput=False)
    mask01 = nc.declare_dram_parameter("mask01", [P, P], bf16, isOutput=False)
    out = nc.declare_dram_parameter("out", [S, DM], bf16, isOutput=True)

    with tile.TileContext(nc) as tc:
        with (
            tc.tile_pool(name="persist", bufs=1) as persist,
            tc.tile_pool(name="small", bufs=3) as small,
        ):
            # ---- persistent tiles
            qTr = persist.tile([P, NH_CORE, S], bf16, tag="qTr")
            kTr = persist.tile([P, 2, S], bf16, tag="kTr")
            vr = persist.tile([P, N_KT, NH_CORE, HD + 1], bf16, tag="vr")
            woTr = persist.tile([P, 2, DM], bf16, tag="woTr")
            aTr = persist.tile([P, 2, S], bf16, tag="aTr")
            mask_sb = persist.tile([P, P], bf16, tag="mask01")

            # ---- phase 0+1: load x/weights, projections.
            with tc.tile_pool(name="xw", bufs=1) as xw:
                # slab-major so each slab DMA writes one contiguous
                # 16KB-per-partition block (hardware-dynamic descriptors)
                xTr = xw.tile([P, N_QC, KO, QC], bf16, tag="xTr")
                wts = {}
                for name, dram in (("q", wqT), ("k", wkT), ("v", wvT)):
                    wts[name] = xw.tile(
                        [P, KO, HSL], bf16, tag=f"w{name}r", name=f"w{name}r"
                    )
                # critical pieces (wq + x slab0) first, 2-ko granularity
                # (2KB lines): wq on gpsimd, slab0 pieces on the HWDGE
                # queues; ko-outer consumption only needs piece k+1 every
                # ~0.9us once piece k has landed.
                nc.sync.dma_start(wts["q"][:, 0:1, :], wqT[:, 0:1, :])
                nc.scalar.dma_start(xTr[:, 0, 0:1, :], xS[0, :, 0:1, :])
                nc.gpsimd.dma_start(wts["k"][:, 0:1, :], wkT[:, 0:1, :])
                nc.sync.dma_start(xTr[:, 0, 1:2, :], xS[0, :, 1:2, :])
                nc.scalar.dma_start(wts["q"][:, 1:3, :], wqT[:, 1:3, :])
                nc.gpsimd.dma_start(wts["k"][:, 1:3, :], wkT[:, 1:3, :])
                nc.sync.dma_start(xTr[:, 0, 2:4, :], xS[0, :, 2:4, :])
                nc.scalar.dma_start(wts["q"][:, 3:6, :], wqT[:, 3:6, :])
                nc.gpsimd.dma_start(wts["k"][:, 3:6, :], wkT[:, 3:6, :])
                nc.sync.dma_start(xTr[:, 0, 4:6, :], xS[0, :, 4:6, :])
                nc.scalar.dma_start(wts["q"][:, 6:8, :], wqT[:, 6:8, :])
                nc.gpsimd.dma_start(wts["k"][:, 6:8, :], wkT[:, 6:8, :])
                nc.sync.dma_start(xTr[:, 0, 6:8, :], xS[0, :, 6:8, :])
                nc.gpsimd.dma_start(xTr[:, 1, 0:4, :], xS[1, :, 0:4, :])
                nc.sync.dma_start(xTr[:, 1, 4:8, :], xS[1, :, 4:8, :])
                nc.scalar.dma_start(wts["v"][:], wvT[:])
                nc.gpsimd.dma_start(xTr[:, 2], xS[2])
                nc.scalar.dma_start(woTr[:], woT[:])
                nc.sync.dma_start(xTr[:, 3], xS[3])
                nc.sync.dma_start(mask_sb[:], mask01[:])
                nc.vector.memset(
                    vr[:, :, :, HD].rearrange("p a b -> p (a b)"), 1.0
                )
                # zero the pad halves of qTr: even heads live on partitions
                # 0:64, odd heads on 64:128 (the other half multiplies the
                # co-resident head's k rows, so it must be zero)
                zeros_f = persist.tile([P, 1], bf16, tag="zeros")
                nc.vector.memset(zeros_f[:], 0.0)
                nc.vector.tensor_copy(
                    qTr[HD:P, 0::2, :],
                    zeros_f[HD:P, 0:1, None].to_broadcast([HD, 2, S]),
                )
                nc.vector.tensor_copy(
                    qTr[0:HD, 1::2, :],
                    zeros_f[0:HD, 0:1, None].to_broadcast([HD, 2, S]),
                )

                # ---- projection helpers (pool-parametrized so phase B can
                # rerun them against 1-bank pools while attention owns PSUM)
                def qk_groups(sl, pool, tags4):
                    # tags4 (phase A): 4 PSUM groups live, ko-outer — each
                    # arriving x ko-piece feeds 4 matmuls, so the stream
                    # tolerates DMA latency. Phase B (1 bank): ko-inner.
                    ssl = slice(sl * QC, (sl + 1) * QC)
                    outs = (("q", 0), ("q", 1), ("k", 0), ("k", 1))
                    pss = {}
                    for name, mt in outs:
                        tg = f"pg{name}{mt}" if tags4 else "pg"
                        pss[(name, mt)] = pool.tile(
                            [P, QC], f32, tag=tg, name=tg
                        )
                        if not tags4:
                            for ko in range(KO):
                                nc.tensor.matmul(
                                    pss[(name, mt)][:],
                                    wts[name][:, ko, mt * P : (mt + 1) * P],
                                    xTr[:, sl, ko, :],
                                    start=(ko == 0),
                                    stop=(ko == KO - 1),
                                )
                            _qk_copy(name, mt, pss[(name, mt)], ssl)
                    if tags4:
                        for ko in range(KO):
                            for name, mt in outs:
                                nc.tensor.matmul(
                                    pss[(name, mt)][:],
                                    wts[name][:, ko, mt * P : (mt + 1) * P],
                                    xTr[:, sl, ko, :],
                                    start=(ko == 0),
                                    stop=(ko == KO - 1),
                                )
                        for name, mt in outs:
                            _qk_copy(name, mt, pss[(name, mt)], ssl)

                def _qk_copy(name, mt, ps, ssl):
                    if name == "k":
                        nc.vector.tensor_copy(kTr[:, mt, ssl], ps[:])
                    else:
                        nc.vector.tensor_copy(
                            qTr[0:HD, 2 * mt, ssl], ps[0:HD, :]
                        )
                        nc.vector.tensor_copy(
                            qTr[HD:P, 2 * mt + 1, ssl], ps[HD:P, :]
                        )

                def v_group(sl, st4, pool, eng):
                    wr = wts["v"]
                    st = 4 * sl + st4
                    ps = pool.tile([P, HSL], f32, tag="pv")
                    for ko in range(KO):
                        nc.tensor.matmul(
                            ps[:],
                            xTr[:, sl, ko, st4 * P : (st4 + 1) * P],
                            wr[:, ko, :],
                            start=(ko == 0),
                            stop=(ko == KO - 1),
                        )
                    if eng == "v":
                        nc.vector.tensor_copy(
                            vr[:, st, :, 0:HD],
                            ps[:].rearrange("p (h d) -> p h d", d=HD),
                        )
                    else:
                        nc.scalar.activation(
                            vr[:, st, :, 0:HD],
                            ps[:].rearrange("p (h d) -> p h d", d=HD),
                            mybir.ActivationFunctionType.Copy,
                        )

                # ---- phase A: slabs 0-1 projections, full PSUM freedom
                with (
                    tc.tile_pool(name="ps_qkA", bufs=1, space="PSUM") as pqa,
                    tc.tile_pool(name="ps_vA", bufs=4, space="PSUM") as pva,
                ):
                    for sl in (0, 1):
                        qk_groups(sl, pqa, True)
                        for st4 in range(4):
                            v_group(sl, st4, pva, "s")

                # ---- attention machinery (phases B and C share these)
                with (
                    tc.tile_pool(name="expr", bufs=5) as expr,
                    tc.tile_pool(name="ps_s", bufs=2, space="PSUM") as ps_s,
                    tc.tile_pool(name="ps_ot", bufs=1, space="PSUM") as ps_ot,
                ):
                    pend = []
                    mul_q = []

                    def normalize(h, qc, ps_acc):
                        # copy the accumulator out first: releases the PSUM
                        # bank quickly (the next head's O matmuls reuse the
                        # tag ~3 iterations later). The sums row goes to
                        # partition 0 separately: the custom-DVE reciprocal
                        # produces garbage on partition-offset inputs.
                        hm, hb = h // 2, (h % 2) * HD
                        sums = small.tile([1, QC], f32, tag="sums", name="sums")
                        nc.vector.tensor_copy(sums[:], ps_acc[HD : HD + 1, :])
                        oc = small.tile([HD, QC], f32, tag="oc", name="oc")
                        nc.vector.tensor_copy(oc[:], ps_acc[0:HD, :])
                        recip = small.tile([1, QC], f32, tag="recip", name="recip")
                        nc.vector.reciprocal_approx_fast(recip[:], sums[:])
                        bcast = small.tile([HD, QC], f32, tag="bcast", name="bcast")
                        nc.gpsimd.partition_broadcast(bcast[:], recip[:])

                        def mul():
                            # deferred ~2 kt iterations: by then the gpsimd
                            # broadcast is done, so this never parks at the
                            # head of the DVE FIFO blocking later copies
                            nc.vector.tensor_mul(
                                aTr[hb : hb + HD, hm, qc * QC : (qc + 1) * QC],
                                oc[:],
                                bcast[:],
                            )

                        mul_q.append(mul)

                    def o_group(h, okt, segs, er_g, ps_ots):
                        for qc, c0, o0, w in reversed(segs):
                            nc.tensor.matmul(
                                ps_ots[qc][:, o0:QC],
                                vr[:, okt, h, :],
                                er_g[:, c0 : c0 + w],
                                start=(okt == 0),
                                stop=(okt == 4 * qc + 3),
                            )
                            if okt == 4 * qc + 3:
                                normalize(h, qc, ps_ots[qc])

                    def flush_one():
                        okt, oh, osegs, oer, ops_ots = pend.pop(0)
                        o_group(oh, okt, osegs, oer, ops_ots)

                    def attn_step(pr, h, kt, ps_ots):
                        qcs = (2 * pr, 2 * pr + 1)
                        hm = h // 2
                        jd = kt // 4
                        off = (kt % 4) * P
                        live = [qc for qc in qcs if qc >= jd]
                        ps_g = ps_s.tile([P, 2 * QC], f32, tag="ps_s", name="ps_g")
                        er_g = expr.tile([P, 2 * QC], bf16, tag="er", name="er_g")
                        segs = []
                        mask_c0 = None
                        for qc in live:
                            diag = qc == jd
                            o0 = off if diag else 0
                            c0 = (qc - qcs[0]) * QC + o0
                            w = QC - o0
                            nc.tensor.matmul(
                                ps_g[:, c0 : c0 + w],
                                kTr[:, hm, kt * P : (kt + 1) * P],
                                qTr[:, h, qc * QC + o0 : (qc + 1) * QC],
                                start=True,
                                stop=True,
                            )
                            if diag:
                                mask_c0 = c0
                            segs.append((qc, c0, o0, w))
                        g0 = segs[0][1]
                        g1 = segs[-1][1] + segs[-1][3]
                        nc.scalar.activation(
                            er_g[:, g0:g1],
                            ps_g[:, g0:g1],
                            mybir.ActivationFunctionType.Exp,
                            scale=EXP_SCALE,
                        )
                        if mask_c0 is not None:
                            # causal mask applied post-exp on the 128
                            # diagonal columns (DVE; keeps the PE free of
                            # the per-diagonal ident LDWEIGHTS + matmul)
                            nc.vector.tensor_mul(
                                er_g[:, mask_c0 : mask_c0 + P],
                                er_g[:, mask_c0 : mask_c0 + P],
                                mask_sb[:],
                            )
                        pend.append((kt, h, segs, er_g, ps_ots))
                        if len(pend) >= PEND:
                            flush_one()
                        if len(mul_q) >= 2:
                            mul_q.pop(0)()

                    PEND = 4  # S->exp->O pipeline depth

                    # ---- phase B: pr0 attention with slab 2-3 projections
                    # interleaved into the PE stream (ACT is exp-bound here,
                    # PE has ~35% slack; proj PSUM shrinks to 2 banks)
                    with (
                        tc.tile_pool(name="ps_qkB", bufs=1, space="PSUM") as pqb,
                        tc.tile_pool(name="ps_vB", bufs=1, space="PSUM") as pvb,
                    ):
                        projB = [
                            lambda: qk_groups(2, pqb, False),
                        ]
                        for st4 in range(4):
                            projB.append(
                                lambda st4=st4: v_group(2, st4, pvb, "v")
                            )
                        projB.append(lambda: qk_groups(3, pqb, False))
                        gi = 0
                        for h in range(NH_CORE):
                            ps_ots = {
                                qc: ps_ot.tile(
                                    [HD + 1, QC], f32,
                                    tag=f"ot{qc % 2}", name="ps_ot",
                                )
                                for qc in (0, 1)
                            }
                            for kt in range(8):
                                attn_step(0, h, kt, ps_ots)
                                # ~one proj unit per 2 kt steps
                                if gi % 2 == 1 and projB:
                                    projB.pop(0)()
                                gi += 1
                        while projB:
                            projB.pop(0)()

                    # head 0 of pr1 carries slab3's v projections as PE
                    # filler (this head is exp-bound); its o_proj chunks
                    # shift to later heads' windows. Runs before ps_o opens
                    # so the 1-bank v pool fits.
                    with tc.tile_pool(
                        name="ps_v3", bufs=1, space="PSUM"
                    ) as pv3:
                        vu = [
                            (lambda st4=st4: v_group(3, st4, pv3, "v"))
                            for st4 in range(4)
                        ]
                        ps_ots = {
                            qc: ps_ot.tile(
                                [HD + 1, QC], f32,
                                tag=f"ot{qc % 2}", name="ps_ot",
                            )
                            for qc in (2, 3)
                        }
                        for kt in range(16):
                            attn_step(1, 0, kt, ps_ots)
                            if kt % 2 == 1 and vu:
                                vu.pop(0)()

                    # ---- phase C: pr1 attention + o_proj (pr0 chunks slot
                    # into head starts, pr1 chunks in the tail)
                    with (
                        tc.tile_pool(name="ps_o", bufs=2, space="PSUM") as ps_o,
                        tc.tile_pool(name="outp", bufs=4) as outp,
                    ):

                        def o_chunk(st, engs):
                            # full o_proj row block for seq tile st: two PSUM
                            # halves, one [128, 1024] SBUF tile
                            ot = outp.tile([P, DM], bf16, tag="ot", name="ot")
                            for nch in range(2):
                                ps = ps_o.tile([P, QC], f32, tag="po", name="po")
                                for kt2 in range(2):
                                    nc.tensor.matmul(
                                        ps[:],
                                        aTr[:, kt2, st * P : (st + 1) * P],
                                        woTr[:, kt2, nch * QC : (nch + 1) * QC],
                                        start=(kt2 == 0),
                                        stop=(kt2 == 1),
                                    )
                                dst = ot[:, nch * QC : (nch + 1) * QC]
                                if engs[nch] == "v":
                                    nc.vector.tensor_copy(dst, ps[:])
                                else:
                                    nc.scalar.activation(
                                        dst, ps[:],
                                        mybir.ActivationFunctionType.Copy,
                                    )
                            # partition-split out DMA over a rotating queue
                            # pair: chunks drain in parallel with 2KB rows
                            qa, qb = [
                                (nc.sync, nc.scalar),
                                (nc.gpsimd, nc.sync),
                                (nc.scalar, nc.gpsimd),
                            ][st % 3]
                            qa.dma_start(
                                out[st * P : st * P + 64, :], ot[0:64, :]
                            )
                            qb.dma_start(
                                out[st * P + 64 : (st + 1) * P, :], ot[64:128, :]
                            )


                        CH = {1: (0, 1, 2), 2: (3, 4, 5), 3: (6, 7)}
                        for h in range(1, NH_CORE):
                            if h == 1:
                                # pr0's deferred aTr muls must land before
                                # its o_proj chunks read aTr
                                while mul_q:
                                    mul_q.pop(0)()
                            # pr0's aTr is complete: slot its o_proj into
                            # this head's PE stream (fills the window while
                            # the previous head's accumulators normalize)
                            for st in CH[h]:
                                o_chunk(st, ("v", "v"))
                            ps_ots = {
                                qc: ps_ot.tile(
                                    [HD + 1, QC], f32,
                                    tag=f"ot{qc % 2}", name="ps_ot",
                                )
                                for qc in (2, 3)
                            }
                            for kt in range(16):
                                attn_step(1, h, kt, ps_ots)
                                if h == 3 and kt == 14:
                                    # all qc2 accumulators are normalized
                                    # by now: land their muls and start
                                    # the o_proj tail early
                                    while mul_q:
                                        mul_q.pop(0)()
                                    o_chunk(8, ("v", "v"))
                                if h == 3 and kt == 15:
                                    o_chunk(9, ("v", "v"))
                        while pend:
                            flush_one()
                        while mul_q:
                            mul_q.pop(0)()

                        # ---- tail: o_proj for pr1's seq tiles (ACT is idle
                        # now, so split the copies between DVE and ACT)
                        for st in range(10, N_KT):
                            o_chunk(st, ("v", "s") if st % 2 == 0 else ("s", "v"))

    nc.compile()
    return nc


def _make_mask01():
    # mask01[p, j] = 0 where key p > query j (future), else 1; multiplies
    # the exp'd diagonal block so masked entries become exact zeros.
    p = np.arange(P)[:, None]
    j = np.arange(P)[None, :]
    return np.where(p > j, np.float32(0.0), np.float32(1.0)).astype(
        ml_dtypes.bfloat16
    )


def make_in_maps(x, Wq, Wk, Wv, Wo):
    mask01 = _make_mask01()

    def wtile(wT, dt=ml_dtypes.bfloat16, scale=1.0):
        # [1024, 256] -> [128, 8, 256] (p, ko, m)
        return np.ascontiguousarray(
            wT.reshape(KO, P, HSL).transpose(1, 0, 2) * scale
        ).astype(dt)

    in_maps = []
    for c in range(8):
        bi, g = c // 4, c % 4
        sl = slice(g * HSL, (g + 1) * HSL)
        # xS[sl, p, ko, s] = x[bi][sl*512+s, ko*128+p]
        xs = np.ascontiguousarray(
            x[bi].reshape(N_QC, QC, KO, P).transpose(0, 3, 2, 1)
        ).astype(ml_dtypes.bfloat16)
        in_maps.append(
            {
                "xS": xs,
                "wqT": wtile(Wq[sl, :].T),
                "wkT": wtile(Wk[sl, :].T),
                "wvT": wtile(Wv[sl, :].T),
                "woT": np.ascontiguousarray(
                    Wo[:, sl].T.reshape(2, P, DM).transpose(1, 0, 2)
                ).astype(ml_dtypes.bfloat16),
                "mask01": mask01,
            }
        )
    return in_maps


def kernel(x, Wq, Wk, Wv, Wo):
    x = np.asarray(x, dtype=np.float32)
    Wq = np.asarray(Wq, dtype=np.float32)
    Wk = np.asarray(Wk, dtype=np.float32)
    Wv = np.asarray(Wv, dtype=np.float32)
    Wo = np.asarray(Wo, dtype=np.float32)
    b, s, dm = x.shape
    assert (b, s, dm) == (2, S, DM), (b, s, dm)

    if "nc" not in _CACHED:
        _CACHED["nc"] = build_program()
    nc = _CACHED["nc"]

    in_maps = make_in_maps(x, Wq, Wk, Wv, Wo)
    res = run_bass_kernel_spmd(nc, in_maps, core_ids=list(range(8)))

    out = np.zeros((2, S, DM), dtype=np.float32)
    for c in range(8):
        out[c // 4] += np.asarray(res.results[c]["out"]).astype(np.float32)
    return out
